# revision 13
# baseline (speedup 1.0000x reference)
"""2-layer GAT (graph attention) forward, distributed across 8 Trainium2 NeuronCores.

Sharding: edges are partitioned by destination-node range (host-side sort by dst);
each core owns N/8 destination nodes and all edges pointing into them, so the
segment softmax and the scatter-add reduction are core-local.  Node features are
computed data-parallel over the owning core's nodes and replicated via AllGather.
Per-edge message aggregation is done as gather (indirect DMA of h[src] rows) +
one-hot matmul scatter into PSUM.  Only the tiny per-graph pooling sums need an
AllReduce at the end.

Implementation notes:
 - features are stored c-major ([c*H+h] instead of [h*C+c], via host-permuted
   weight matrices) so the per-edge softmax-weight broadcast multiply has a
   unit-stride innermost dim on every operand (DVE 2x mode).
 - attention logits q = a_e + a_s[src] + a_d[dst] are accumulated in PSUM by
   matmuls (edge_attr x Ve, identity x gathered a_s, onehot^T x a_d) for a
   whole dst tile, then exp(leaky_relu(q)) = max(exp(q), exp(0.2 q)) is
   evaluated in two batched ACT ops per tile.
 - gather rows are [h (c-major) | a_s | pad] so a single indirect DMA per tile
   fetches everything keyed by src.

kernel(**inputs) accepts the full (unsharded) inputs of reference.setup_inputs()
and returns the full [G, CLS] log-softmax output.
"""
import sys
sys.path.insert(0, "/opt/trn_rl_repo")

import numpy as np
import ml_dtypes

import concourse.bacc as bacc
import concourse.bass as bass
import concourse.mybir as mybir
import concourse.tile as tile
from concourse import library_config
from concourse.bass_utils import run_bass_kernel_spmd

F32 = mybir.dt.float32
BF16 = mybir.dt.bfloat16
I16 = mybir.dt.int16
NEG_ATT = 0.2
NEG_ACT = 0.01
AF = mybir.ActivationFunctionType
OP = mybir.AluOpType


def cdiv(a, b):
    return (a + b - 1) // b


class Cfg:
    def __init__(self, N=20000, E=320000, F_IN=256, ED=16, H=8, C=64, G=64,
                 CLS=10, ncores=8, fdt="bf16"):
        self.N, self.E, self.F_IN, self.ED = N, E, F_IN, ED
        self.H, self.C, self.G, self.CLS = H, C, G, CLS
        self.ncores = ncores
        self.HC = H * C
        self.NPC = cdiv(N, ncores)            # real nodes per core
        self.NT = cdiv(self.NPC, 128)         # 128-node dst tiles per core
        self.NPCP = self.NT * 128             # padded nodes per core
        self.NP = ncores * self.NPCP          # padded global nodes
        assert self.NP < 32768, "gather indices must fit int16"
        assert F_IN % 128 == 0 and self.HC % 128 == 0
        self.KX = F_IN // 128
        self.KH = self.HC // 128
        self.fdt = fdt
        self.FDT = BF16 if fdt == "bf16" else F32
        self.npdt = ml_dtypes.bfloat16 if fdt == "bf16" else np.float32
        # h-row in the gathered table: [h (HC) | a_s (H) | pad]; bytes % 256 == 0
        align = 128 if fdt == "bf16" else 64
        self.HROW = cdiv(self.HC + self.H, align) * align
        # c-major feature permutation: new col f = c*H+h holds old col h*C+c
        self.cm = (np.arange(self.HC) % H) * C + (np.arange(self.HC) // H)


def _wrap16(arr_i16, nch_list):
    """Per-tile wrap of an int16 index list into the [128, n/16] SWDGE layout."""
    cols = []
    off = 0
    for nch in nch_list:
        n = nch * 128
        seg = arr_i16[off:off + n]
        cols.append(np.tile(seg.reshape(n // 16, 16).T, (8, 1)))
        off += n
    return np.ascontiguousarray(np.concatenate(cols, axis=1))


def preprocess(cfg, inputs):
    """Host-side sharding: sort edges by dst, bucket into per-core dst tiles,
    build index/one-hot helper tables.  All floating-point math stays on device."""
    c = cfg
    x = np.asarray(inputs["x"], np.float32)
    ei = np.asarray(inputs["edge_index"]).astype(np.int64)
    ea = np.asarray(inputs["edge_attr"], np.float32)
    batch = np.asarray(inputs["batch"]).astype(np.int64)
    src, dst = ei[0], ei[1]

    order = np.argsort(dst, kind="stable")
    ds, ss, eap = dst[order], src[order], ea[order]
    core = ds // c.NPC
    local = ds - core * c.NPC
    tilei = local // 128
    dstl = local % 128

    cnt = np.zeros((c.ncores, c.NT), np.int64)
    np.add.at(cnt, (core, tilei), 1)
    nch = np.maximum(1, (cnt.max(axis=0) + 127) // 128)   # chunks per tile slot
    NCH = int(nch.sum())
    TOT = NCH * 128
    slot0 = np.concatenate([[0], np.cumsum(nch)])

    starts = np.concatenate([[0], np.cumsum(cnt.reshape(-1))])

    def gid(n):
        cc = n // c.NPC
        return cc * c.NPCP + (n - cc * c.NPC)

    per_core = []
    for cc in range(c.ncores):
        srcidx = np.zeros(TOT, np.int16)
        dstlv = np.full(TOT, 200.0, np.float32)
        ea_rows = np.zeros((TOT, c.ED), np.float32)
        for t in range(c.NT):
            k = cc * c.NT + t
            s, e = starts[k], starts[k + 1]
            n = e - s
            o = slot0[t] * 128
            if n:
                srcidx[o:o + n] = gid(ss[s:e]).astype(np.int16)
                dstlv[o:o + n] = dstl[s:e].astype(np.float32)
                ea_rows[o:o + n] = eap[s:e]
        srcidx_w = _wrap16(srcidx, nch)                      # [128, NCH*8]
        dstl_t = np.concatenate(
            [dstlv[slot0[t] * 128:slot0[t + 1] * 128].reshape(nch[t], 128).T
             for t in range(c.NT)], axis=1)                  # [128, NCH]
        dstlrep = np.tile(dstlv, (128, 1))                   # [128, TOT]
        eaT = np.ascontiguousarray(ea_rows.T)                # [ED, TOT]

        nreal = min(c.NPC, c.N - cc * c.NPC)
        xt = np.zeros((c.F_IN, c.NPCP), np.float32)
        xt[:, :nreal] = x[cc * c.NPC: cc * c.NPC + nreal].T
        goneh = np.zeros((c.NPCP, c.G), np.float32)
        bslice = batch[cc * c.NPC: cc * c.NPC + nreal]
        goneh[np.arange(nreal), bslice] = 1.0

        per_core.append(dict(
            xt=xt.astype(c.npdt),
            srcidx=srcidx_w,
            dstl=dstl_t.astype(np.float32),
            dstlrep=dstlrep.astype(c.npdt),
            eat=eaT.astype(c.npdt),
            goneh=goneh.astype(c.npdt),
        ))

    gcnt = np.bincount(batch, minlength=c.G).astype(np.float32)
    invcnt = (1.0 / np.maximum(gcnt, 1.0)).reshape(c.G, 1).astype(np.float32)

    def rep(v, rows, perm=None):
        v = np.asarray(v, np.float32).reshape(1, -1)
        if perm is not None:
            v = v[:, perm]
        return np.tile(v, (rows, 1))

    cm = c.cm
    w0 = np.asarray(inputs["W0"], np.float32)[:, cm]
    w1 = np.asarray(inputs["W1"], np.float32)[cm][:, cm]
    wlin = np.asarray(inputs["Wlin"], np.float32)[cm, :]

    shared = dict(
        w0=w0.astype(c.npdt),
        w1=w1.astype(c.npdt),
        wlin=wlin.astype(c.npdt),
        blin=np.asarray(inputs["blin"], np.float32).reshape(1, c.CLS).astype(c.npdt),
        b0r=rep(inputs["b0"], 128, cm).astype(c.npdt),
        b1r=rep(inputs["b1"], 128, cm).astype(c.npdt),
        atts0=rep(inputs["att_src0"], 128, cm).astype(c.npdt),
        atd0=rep(inputs["att_dst0"], 128, cm).astype(c.npdt),
        atts1=rep(inputs["att_src1"], 128, cm).astype(c.npdt),
        atd1=rep(inputs["att_dst1"], 128, cm).astype(c.npdt),
        ate0=rep(inputs["att_edge0"], c.ED),
        ate1=rep(inputs["att_edge1"], c.ED),
        we0=np.asarray(inputs["We0"], np.float32),
        we1=np.asarray(inputs["We1"], np.float32),
        eye=np.eye(128, dtype=np.float32).astype(c.npdt),
        iota=np.tile(np.arange(128, dtype=np.float32), (128, 1)).astype(c.npdt),
        iotac=np.arange(128, dtype=np.float32).reshape(128, 1),
        ones1=np.ones((1, 128), np.float32).astype(c.npdt),
        invcnt=invcnt,
    )
    in_maps = [{**pc, **shared} for pc in per_core]
    return in_maps, [int(v) for v in nch]


def build(cfg, nch, collectives=True, repeat=1):
    c = cfg
    FDT = c.FDT
    H = c.H
    NCH = sum(nch)
    TOT = NCH * 128
    slot0 = np.concatenate([[0], np.cumsum(nch)]).astype(int)
    rg = [list(range(c.ncores))]

    nc = bacc.Bacc("TRN2", target_bir_lowering=False, debug=False,
                   num_devices=c.ncores)

    def EI(name, shape, dt):
        return nc.dram_tensor(name, list(shape), dt, kind="ExternalInput")

    xt_e = EI("xt", (c.F_IN, c.NPCP), FDT)
    srcidx_e = EI("srcidx", (128, NCH * 8), I16)
    dstl_e = EI("dstl", (128, NCH), F32)
    dstlrep_e = EI("dstlrep", (128, TOT), FDT)
    eat_e = EI("eat", (c.ED, TOT), FDT)
    goneh_e = EI("goneh", (c.NPCP, c.G), FDT)
    w0_e = EI("w0", (c.F_IN, c.HC), FDT)
    w1_e = EI("w1", (c.HC, c.HC), FDT)
    wlin_e = EI("wlin", (c.HC, c.CLS), FDT)
    blin_e = EI("blin", (1, c.CLS), FDT)
    b0r_e = EI("b0r", (128, c.HC), FDT)
    b1r_e = EI("b1r", (128, c.HC), FDT)
    atts0_e = EI("atts0", (128, c.HC), FDT)
    atd0_e = EI("atd0", (128, c.HC), FDT)
    atts1_e = EI("atts1", (128, c.HC), FDT)
    atd1_e = EI("atd1", (128, c.HC), FDT)
    ate0_e = EI("ate0", (c.ED, c.HC), F32)
    ate1_e = EI("ate1", (c.ED, c.HC), F32)
    we0_e = EI("we0", (c.ED, c.HC), F32)
    we1_e = EI("we1", (c.ED, c.HC), F32)
    eye_e = EI("eye", (128, 128), FDT)
    iota_e = EI("iota", (128, 128), FDT)
    iotac_e = EI("iotac", (128, 1), F32)
    ones1_e = EI("ones1", (1, 128), FDT)
    invcnt_e = EI("invcnt", (c.G, 1), F32)

    out_e = nc.dram_tensor("out", [c.G, c.CLS], F32, kind="ExternalOutput")

    def cmaj(ap):
        """view a [128, HC] c-major AP as [128, C, H] (innermost unit-stride)"""
        return ap.rearrange("p (cc h) -> p cc h", h=H)

    def hview(ap):
        """view a [128, HC] c-major AP as [128, H, C] (strided head-major)"""
        return ap.rearrange("p (cc h) -> p h cc", h=H)

    with tile.TileContext(nc, num_cores=c.ncores) as tc:
        import contextlib
        with contextlib.ExitStack() as stack:
            cpool = stack.enter_context(tc.tile_pool(name="consts", bufs=1))
            dram = stack.enter_context(tc.tile_pool(name="dram", bufs=1, space="DRAM"))
            ppool = stack.enter_context(tc.tile_pool(name="ppersist", bufs=1, space="PSUM"))

            nc.gpsimd.load_library(library_config.mlp)

            def load_const(ext, shape, dt, name):
                tl = cpool.tile(list(shape), dt, tag=name)
                nc.sync.dma_start(tl[:], ext[:])
                return tl

            eye = load_const(eye_e, (128, 128), FDT, "eye")
            iota = load_const(iota_e, (128, 128), FDT, "iota")
            iotac = load_const(iotac_e, (128, 1), F32, "iotac")
            ones1 = load_const(ones1_e, (1, 128), FDT, "ones1")
            blin = load_const(blin_e, (1, c.CLS), FDT, "blin")
            b0r = load_const(b0r_e, (128, c.HC), FDT, "b0r")
            b1r = load_const(b1r_e, (128, c.HC), FDT, "b1r")
            atts0 = load_const(atts0_e, (128, c.HC), FDT, "atts0")
            atd0 = load_const(atd0_e, (128, c.HC), FDT, "atd0")
            atts1 = load_const(atts1_e, (128, c.HC), FDT, "atts1")
            atd1 = load_const(atd1_e, (128, c.HC), FDT, "atd1")
            invcnt = load_const(invcnt_e, (c.G, 1), F32, "invcnt")

            def load_chunks(ext, kparts, cols, name):
                tiles = []
                for k in range(kparts):
                    tl = cpool.tile([128, cols], FDT, tag=f"{name}{k}")
                    nc.sync.dma_start(tl[:], ext[k * 128:(k + 1) * 128, :])
                    tiles.append(tl)
                return tiles

            w0c = load_chunks(w0_e, c.KX, c.HC, "w0")
            w1c = load_chunks(w1_e, c.KH, c.HC, "w1")
            wlc = load_chunks(wlin_e, c.KH, c.CLS, "wl")

            # Ve[l] = contract(We[l], att_edge[l]) over C  -> [ED, H]
            def make_ve(we_ext, ate_ext, name):
                wet = cpool.tile([c.ED, c.H, c.C], F32, tag=name + "w")
                nc.sync.dma_start(wet[:], we_ext.ap().rearrange("d (h cc) -> d h cc", h=H))
                atet = cpool.tile([c.ED, c.H, c.C], F32, tag=name + "a")
                nc.sync.dma_start(atet[:], ate_ext.ap().rearrange("d (h cc) -> d h cc", h=H))
                prod = cpool.tile([c.ED, c.H, c.C], F32, tag=name + "p")
                nc.vector.tensor_tensor(prod[:], wet[:], atet[:], OP.mult)
                ve32 = cpool.tile([c.ED, c.H], F32, tag=name + "3")
                nc.vector.tensor_reduce(ve32[:], prod[:], axis=mybir.AxisListType.X,
                                        op=OP.add)
                vef = cpool.tile([c.ED, c.H], FDT, tag=name)
                nc.vector.tensor_copy(vef[:], ve32[:])
                return vef

            ve0 = make_ve(we0_e, ate0_e, "ve0")
            ve1 = make_ve(we1_e, ate1_e, "ve1")

            h0_loc = dram.tile([c.NPCP, c.HROW], FDT)
            h0_full = dram.tile([c.NP, c.HROW], FDT, addr_space="Shared")
            h1_loc = dram.tile([c.NPCP, c.HROW], FDT)
            h1_full = dram.tile([c.NP, c.HROW], FDT, addr_space="Shared")
            ad0 = dram.tile([c.NPCP, c.H], FDT)
            ad1 = dram.tile([c.NPCP, c.H], FDT)
            f1t = dram.tile([c.HC, c.NPCP], FDT)
            pool_in = dram.tile([c.G, c.HC], F32)
            pool_out = dram.tile([c.G, c.HC], F32, addr_space="Shared")

            poolP = ppool.tile([c.G, c.HC], F32, tag="poolP")

            # zero the never-written pad columns of the gather tables
            PW = c.HROW - c.HC - c.H
            if PW:
                zpad = cpool.tile([128, PW], FDT, tag="zpad")
                nc.vector.memset(zpad[:], 0.0)
                for hl in (h0_loc, h1_loc):
                    for t in range(c.NT):
                        nc.sync.dma_start(
                            hl[t * 128:(t + 1) * 128, c.HC + c.H:c.HROW], zpad[:])

            # ---- phase: node transform -------------------------------------
            def transform(kx, lhsT_src, w_chunks, atts, atd, h_loc, ad_loc):
                with tc.tile_pool(name="tf", bufs=3) as sb, \
                     tc.tile_pool(name="tfp", bufs=2, space="PSUM") as pp:
                    for t in range(c.NT):
                        hp = pp.tile([128, c.HC], F32, tag="hp")
                        for k in range(kx):
                            lh = sb.tile([128, 128], FDT, tag="lh")
                            nc.sync.dma_start(
                                lh[:], lhsT_src[k * 128:(k + 1) * 128,
                                                t * 128:(t + 1) * 128])
                            nc.tensor.matmul(hp[:], lh[:], w_chunks[k][:],
                                             start=(k == 0), stop=(k == kx - 1))
                        hsb = sb.tile([128, c.HC], FDT, tag="hsb")
                        nc.scalar.copy(hsb[:], hp[:])
                        tmp = sb.tile([128, c.HC], F32, tag="tmp")
                        asad = sb.tile([128, 2 * c.H], F32, tag="asad")
                        nc.vector.tensor_tensor(tmp[:], hsb[:], atts[:], OP.mult)
                        nc.vector.tensor_reduce(asad[:, 0:c.H], hview(tmp[:]),
                                                axis=mybir.AxisListType.X, op=OP.add)
                        nc.vector.tensor_tensor(tmp[:], hsb[:], atd[:], OP.mult)
                        nc.vector.tensor_reduce(asad[:, c.H:2 * c.H], hview(tmp[:]),
                                                axis=mybir.AxisListType.X, op=OP.add)
                        rows = slice(t * 128, (t + 1) * 128)
                        nc.sync.dma_start(h_loc[rows, 0:c.HC], hsb[:])
                        nc.gpsimd.dma_start(h_loc[rows, c.HC:c.HC + c.H],
                                            asad[:, 0:c.H])
                        nc.gpsimd.dma_start(ad_loc[rows, :], asad[:, c.H:2 * c.H])

            # ---- phase: edge message passing -------------------------------
            def message(h_full, ad_loc, ve, brep, consumer):
                with tc.tile_pool(name="mg", bufs=2) as gb, \
                     tc.tile_pool(name="mtile", bufs=2) as tb, \
                     tc.tile_pool(name="ms", bufs=4) as sb, \
                     tc.tile_pool(name="mu", bufs=2, space="PSUM") as pU, \
                     tc.tile_pool(name="md", bufs=1, space="PSUM") as pD, \
                     tc.tile_pool(name="mq", bufs=2, space="PSUM") as pQ, \
                     tc.tile_pool(name="mt", bufs=2, space="PSUM") as pT:
                    for t in range(c.NT):
                        n = nch[t]
                        sidx = tb.tile([128, n * 8], I16, tag="sidx")
                        nc.sync.dma_start(
                            sidx[:], srcidx_e[:, slot0[t] * 8:slot0[t + 1] * 8])
                        dstl = tb.tile([128, n], F32, tag="dstl")
                        nc.sync.dma_start(dstl[:], dstl_e[:, slot0[t]:slot0[t + 1]])
                        dlr = tb.tile([128, n * 128], FDT, tag="dlr")
                        nc.sync.dma_start(
                            dlr[:], dstlrep_e[:, slot0[t] * 128:slot0[t + 1] * 128])
                        eat = tb.tile([c.ED, n * 128], FDT, tag="eat")
                        nc.sync.dma_start(
                            eat[:], eat_e[:, slot0[t] * 128:slot0[t + 1] * 128])
                        adt = tb.tile([128, c.H], FDT, tag="adt")
                        nc.sync.dma_start(adt[:], ad_loc[t * 128:(t + 1) * 128, :])
                        gh = gb.tile([128, n, c.HROW], FDT, tag="gh")
                        nc.gpsimd.dma_gather(gh[:], h_full[:], sidx[:],
                                             n * 128, n * 128, c.HROW,
                                             single_packet=False)
                        # attention logits for the whole tile: q = a_e+a_s+a_d
                        qa = pQ.tile([128, n * H], F32, tag="qa")
                        nc.tensor.matmul(qa[:], eye[:],
                                         gh[:, :, c.HC:c.HC + c.H],
                                         start=True, stop=False,
                                         skip_group_check=True)
                        for ch in range(n):
                            ohT = sb.tile([128, 128], FDT, tag="ohT")
                            nc.vector.tensor_scalar(
                                ohT[:], dlr[:, ch * 128:(ch + 1) * 128],
                                iotac[:], None, OP.is_equal)
                            qs = qa[:, ch * H:(ch + 1) * H]
                            nc.tensor.matmul(qs, eat[:, ch * 128:(ch + 1) * 128],
                                             ve[:], start=False, stop=False,
                                             skip_group_check=True)
                            nc.tensor.matmul(qs, ohT[:], adt[:],
                                             start=False, stop=True,
                                             skip_group_check=True)
                        # exp(leaky_relu(q)) = max(exp(q), exp(0.2 q)), batched
                        e1 = tb.tile([128, n * H], FDT, tag="e1")
                        nc.scalar.activation(e1[:], qa[:], AF.Exp)
                        e2 = tb.tile([128, n * H], FDT, tag="e2")
                        nc.scalar.activation(e2[:], qa[:], AF.Exp, scale=NEG_ATT)
                        exa = tb.tile([128, n * H], FDT, tag="exa")
                        nc.vector.tensor_max(exa[:], e1[:], e2[:])
                        U = pU.tile([128, c.HC], F32, tag="U")
                        D = pD.tile([128, c.H], F32, tag="D")
                        for ch in range(n):
                            oh = sb.tile([128, 128], FDT, tag="oh")
                            nc.vector.tensor_scalar(oh[:], iota[:],
                                                    dstl[:, ch:ch + 1], None,
                                                    OP.is_equal)
                            exh = sb.tile([128, c.C, c.H], FDT, tag="exh")
                            exs = exa[:, ch * H:(ch + 1) * H]
                            nc.vector.tensor_tensor(
                                exh[:], cmaj(gh[:, ch, 0:c.HC]),
                                exs.unsqueeze(1).broadcast_to([128, c.C, c.H]),
                                OP.mult)
                            nc.tensor.matmul(U[:], oh[:], exh[:],
                                             start=(ch == 0), stop=(ch == n - 1))
                            nc.tensor.matmul(D[:], oh[:], exs,
                                             start=(ch == 0), stop=(ch == n - 1))
                        # tile epilogue: out = U / (D + eps) + b
                        rdt = tb.tile([128, c.H], F32, tag="rdt")
                        nc.vector.tensor_single_scalar(rdt[:], D[:], 1e-16, OP.add)
                        rd = tb.tile([128, c.H], F32, tag="rd")
                        nc.vector.reciprocal(rd[:], rdt[:])
                        o1 = tb.tile([128, c.C, c.H], F32, tag="o1")
                        nc.vector.tensor_tensor(
                            o1[:], cmaj(U[:]),
                            rd[:].unsqueeze(1).broadcast_to([128, c.C, c.H]),
                            OP.mult)
                        o2 = tb.tile([128, c.HC], F32, tag="o2")
                        nc.vector.tensor_tensor(
                            o2[:], o1[:].rearrange("p cc h -> p (cc h)"),
                            brep[:], OP.add)
                        consumer(t, o2, tb, pT)

            def consume0(t, o2, sb, pT):
                f1 = sb.tile([128, c.HC], FDT, tag="f1")
                nc.vector.scalar_tensor_tensor(f1[:], o2[:], NEG_ACT, o2[:],
                                               OP.mult, OP.max)
                for k in range(c.KH):
                    tp = pT.tile([128, 128], FDT, tag="tp")
                    nc.tensor.transpose(tp[:], f1[:, k * 128:(k + 1) * 128], eye[:])
                    tpc = sb.tile([128, 128], FDT, tag="tpc")
                    nc.scalar.copy(tpc[:], tp[:])
                    nc.sync.dma_start(
                        f1t[k * 128:(k + 1) * 128, t * 128:(t + 1) * 128], tpc[:])

            def consume1(t, o2, sb, pT):
                h2 = sb.tile([128, c.HC], FDT, tag="h2")
                nc.vector.scalar_tensor_tensor(h2[:], o2[:], NEG_ACT, o2[:],
                                               OP.mult, OP.max)
                go = sb.tile([128, c.G], FDT, tag="go")
                nc.sync.dma_start(go[:], goneh_e[t * 128:(t + 1) * 128, :])
                nc.tensor.matmul(poolP[:], go[:], h2[:],
                                 start=(t == 0), stop=(t == c.NT - 1))

            def allgather(loc, full):
                if collectives:
                    nc.gpsimd.collective_compute("AllGather", OP.bypass,
                                                 ins=[loc.opt()], outs=[full.opt()],
                                                 replica_groups=rg)
                else:  # single-core profiling stand-in
                    nc.sync.dma_start(full[0:c.NPCP, :], loc[:])

            for _rep in range(repeat):
                transform(c.KX, xt_e.ap(), w0c, atts0, atd0, h0_loc, ad0)
                allgather(h0_loc, h0_full)
                message(h0_full, ad0, ve0, b0r, consume0)
                transform(c.KH, f1t[:, :], w1c, atts1, atd1, h1_loc, ad1)
                allgather(h1_loc, h1_full)
                message(h1_full, ad1, ve1, b1r, consume1)

            # ---- pooling + classifier + log_softmax ------------------------
            with tc.tile_pool(name="fin", bufs=2) as sb, \
                 tc.tile_pool(name="finp", bufs=2, space="PSUM") as pp:
                psb = sb.tile([c.G, c.HC], F32, tag="psb")
                nc.scalar.copy(psb[:], poolP[:])
                nc.sync.dma_start(pool_in[:], psb[:])
                if collectives:
                    nc.gpsimd.collective_compute("AllReduce", OP.add,
                                                 ins=[pool_in.opt()],
                                                 outs=[pool_out.opt()],
                                                 replica_groups=rg)
                else:
                    nc.sync.dma_start(pool_out[:], pool_in[:])
                pools = sb.tile([c.G, c.HC], F32, tag="pools")
                nc.sync.dma_start(pools[:], pool_out[:])
                pooled = sb.tile([c.G, c.HC], FDT, tag="pooled")
                nc.scalar.activation(pooled[:], pools[:], AF.Copy,
                                     scale=invcnt[:])
                lg = pp.tile([c.G, c.CLS], F32, tag="lg")
                for k in range(c.KH):
                    tp = pp.tile([128, c.G], FDT, tag="ftp")
                    nc.tensor.transpose(tp[:], pooled[:, k * 128:(k + 1) * 128],
                                        eye[0:c.G, 0:c.G])
                    tpc = sb.tile([128, c.G], FDT, tag="ftpc")
                    nc.scalar.copy(tpc[:], tp[:])
                    nc.tensor.matmul(lg[:], tpc[:], wlc[k][:],
                                     start=(k == 0), stop=False)
                nc.tensor.matmul(lg[:], ones1[0:1, 0:c.G], blin[:],
                                 start=False, stop=True)
                lgs = sb.tile([c.G, c.CLS], F32, tag="lgs")
                nc.scalar.copy(lgs[:], lg[:])
                mx = sb.tile([c.G, 1], F32, tag="mx")
                nc.vector.tensor_reduce(mx[:], lgs[:], axis=mybir.AxisListType.X,
                                        op=OP.max)
                zc = sb.tile([c.G, c.CLS], F32, tag="zc")
                nc.vector.tensor_scalar(zc[:], lgs[:], mx[:], None, OP.subtract)
                ez = sb.tile([c.G, c.CLS], F32, tag="ez")
                se = sb.tile([c.G, 1], F32, tag="se")
                nc.scalar.activation(ez[:], zc[:], AF.Exp, accum_out=se[:])
                lse = sb.tile([c.G, 1], F32, tag="lse")
                nc.scalar.activation(lse[:], se[:], AF.Ln)
                osb = sb.tile([c.G, c.CLS], F32, tag="osb")
                nc.vector.tensor_scalar(osb[:], zc[:], lse[:], None, OP.subtract)
                nc.sync.dma_start(out_e[:], osb[:])

    nc.compile()
    return nc


_CACHE = {}


def _get_program(cfg, nch):
    key = (cfg.N, cfg.E, cfg.F_IN, cfg.ED, cfg.H, cfg.C, cfg.G, cfg.CLS,
           cfg.ncores, cfg.fdt, tuple(nch))
    if key not in _CACHE:
        _CACHE[key] = build(cfg, nch)
    return _CACHE[key]


def run(inputs, cfg=None):
    cfg = cfg or Cfg()
    in_maps, nch = preprocess(cfg, inputs)
    nc = _get_program(cfg, nch)
    res = run_bass_kernel_spmd(nc, in_maps, list(range(cfg.ncores)))
    return res.results[0]["out"].astype(np.float32)


def kernel(**inputs) -> np.ndarray:
    return run(inputs)
